# revision 10
# baseline (speedup 1.0000x reference)
"""Trainium2 Bass kernel for nn_CP_Based (CP-decomposition interaction layer).

Math (full problem):
    t[b,f,r,u] = sum_d X[b,f,d] * K[d,r,f,u]      (B=1024, F=64, D=4, R=32, U=128)
    had[b,r,u] = prod_f t[b,f,r,u]
    out[b,u]   = sum_r had[b,r,u]

Strategy (v2):
  * Feature-tripling (host repack): 21 triples + 1 padded single = 22
    factor planes, K=64 contraction each (d3 = 4^3).
  * Sharding 2x4 (batch/2 x units/4): 8.7MB HBM per core, PSUM pair-tiles
    small enough to double-buffer.
  * float32r matmuls (1 cycle/row); factor pairs co-execute on disjoint
    PE row-halves (tile_position).
  * PSUM egress is the bottleneck (only DVE and Act can read PSUM; Pool
    cannot; DMA cannot). Per batch-tile the 22 planes land in 11 PSUM
    pair-tiles [128, 2048] and are consumed by two lanes:
      - DVE pair-reduce: tensor_reduce(op=mult) over the pair axis ->
        one bf16 product plane per pair-tile,
      - Act pair-copy: one ACTIVATE moves both planes -> SBUF bf16; a
        cheap bf16 tensor_tensor merges the two halves.
    The remaining product tree runs in bf16 SBUF split between Pool and
    DVE (2x mode). Final sum over r is a contiguous innermost reduce
    (u-major, r-inner column order).
"""

import numpy as np

B, F, D, R, U = 1024, 64, 4, 32, 128
NCORES = 8
NB, NU = 2, 4                # core grid: 2 batch shards x 4 unit shards
BLOC = B // NB               # 512 batch rows per core
NBT = BLOC // 128            # 4 partition tiles of batch
ULOC = U // NU               # 32 units per core
RULOC = ULOC * R             # 1024 columns per core (u-major, r-inner)
NT = 21                      # feature triples
NFAC = 22                    # 21 triples + 1 padded single
NGRP = NFAC // 2             # 11 pairs of row-tiled factors
D3 = 64                      # contraction dim per triple

# Tunables: which pair-tiles the DVE consumes via pair-reduce (rest go to
# Act pair-copy), and what fraction of bf16 merges Pool takes.
PAIRS_DVE = (0, 3, 6, 9)
POOL_PAT = (1, 0, 1, 1, 0)   # 1 -> Pool, 0 -> DVE, cycled per merge

_cached = {}


def _build_nc():
    import concourse.bass as bass
    import concourse.mybir as mybir
    import concourse.tile as tile
    from concourse import bacc

    fp32 = mybir.dt.float32
    fp32r = mybir.dt.float32r
    bf16 = mybir.dt.bfloat16
    nc = bacc.Bacc("TRN2", target_bir_lowering=False, debug=False)

    xt_d = nc.dram_tensor("xt", [128, NGRP * BLOC], fp32r, kind="ExternalInput").ap()
    kr_d = nc.dram_tensor("kr", [NGRP, 128, RULOC], fp32r, kind="ExternalInput").ap()
    out_d = nc.dram_tensor("out", [BLOC, ULOC], fp32, kind="ExternalOutput").ap()

    pd_set = set(PAIRS_DVE)

    with tile.TileContext(nc) as tc:
        with (
            tc.tile_pool(name="const", bufs=1) as const_pool,
            tc.tile_pool(name="kt", bufs=NGRP) as kpool,
            tc.tile_pool(name="sc", bufs=4) as spool,
            tc.tile_pool(name="rt", bufs=6) as rpool,
            tc.tile_pool(name="outp", bufs=1) as opool,
            tc.tile_pool(name="ps", bufs=2, space="PSUM") as pspool,
        ):
            xt = const_pool.tile([128, NGRP * BLOC], fp32r)
            nc.sync.dma_start(xt[:], xt_d[:])

            kts = []
            for m in range(NGRP):
                kt = kpool.tile([128, RULOC], fp32r, tag="kt")
                nc.sync.dma_start(kt[:], kr_d[m])
                kts.append(kt)

            osum = opool.tile([128, NBT * ULOC], fp32)

            for bt in range(NBT):
                merge_q = []
                nmerge = 0

                def merge_step():
                    nonlocal nmerge
                    b_ = merge_q.pop()
                    a_ = merge_q.pop()
                    dst = rpool.tile([128, RULOC], bf16, tag="rt")
                    if POOL_PAT[nmerge % len(POOL_PAT)]:
                        nc.gpsimd.tensor_mul(dst[:], a_[:], b_[:])
                    else:
                        nc.vector.tensor_mul(dst[:], a_[:], b_[:])
                    nmerge += 1
                    merge_q.append(dst)

                for m in range(NGRP):
                    kt = kts[m]
                    ps = pspool.tile([128, 2 * RULOC], fp32, tag="ps")
                    # plane 2m -> cols [0, RULOC), plane 2m+1 -> [RULOC, 2*RULOC)
                    for c in range(RULOC // 512):
                        for s in range(2):
                            cs = slice(s * RULOC + c * 512, s * RULOC + (c + 1) * 512)
                            nc.tensor.matmul(
                                ps[:, cs],
                                xt[64 * s : 64 * s + D3,
                                   m * BLOC + bt * 128 : m * BLOC + (bt + 1) * 128],
                                kt[64 * s : 64 * s + D3, c * 512 : (c + 1) * 512],
                                start=True,
                                stop=True,
                                tile_position=(64 * s, 0),
                            )
                    if m in pd_set:
                        # DVE: elementwise product of the two planes via a
                        # strided mult-reduce over the pair axis
                        root = rpool.tile([128, RULOC], bf16, tag="rt")
                        nc.vector.tensor_reduce(
                            root[:],
                            ps[:].rearrange("p (two n) -> p n two", two=2),
                            axis=mybir.AxisListType.X,
                            op=mybir.AluOpType.mult,
                        )
                        merge_q.append(root)
                    else:
                        # Act: single op moves both planes PSUM->SBUF (bf16),
                        # then a cheap bf16 mul of the halves
                        sc = spool.tile([128, 2 * RULOC], bf16, tag="sc")
                        nc.scalar.copy(sc[:], ps[:])
                        root = rpool.tile([128, RULOC], bf16, tag="rt")
                        if POOL_PAT[nmerge % len(POOL_PAT)]:
                            nc.gpsimd.tensor_mul(
                                root[:], sc[:, :RULOC], sc[:, RULOC:]
                            )
                        else:
                            nc.vector.tensor_mul(
                                root[:], sc[:, :RULOC], sc[:, RULOC:]
                            )
                        nmerge += 1
                        merge_q.append(root)
                    while len(merge_q) >= 2 and len(merge_q) % 2 == 0:
                        merge_step()

                while len(merge_q) > 1:
                    merge_step()
                pfin = merge_q.pop()

                nc.vector.tensor_reduce(
                    osum[:, bt * ULOC : (bt + 1) * ULOC],
                    pfin[:].rearrange("p (u r) -> p u r", r=R),
                    axis=mybir.AxisListType.X,
                    op=mybir.AluOpType.add,
                )

            for bt in range(NBT):
                nc.sync.dma_start(
                    out_d[bt * 128 : (bt + 1) * 128, :],
                    osum[:, bt * ULOC : (bt + 1) * ULOC],
                )

    nc.compile()
    return nc


def _host_prep(X, K):
    """Repack inputs per core.

    Factor j < 21 covers features (3j, 3j+1, 3j+2), contraction index
    d3 = 16*d0 + 4*d1 + d2; factor 21 is feature 63 zero-padded.
    Row convention: row = 64*s + d3 holds factor j = 2m+s.
    Column convention (kernel side): col = u_loc*32 + r  (u-major, r-inner).

    xt[core][row, m*BLOC + bt*128 + b]
    kr[uq][m, row, u_loc*32 + r]
    """
    f32 = np.float32
    ia = [3 * j for j in range(NT)]
    ib = [3 * j + 1 for j in range(NT)]
    ic = [3 * j + 2 for j in range(NT)]

    ka = K[:, :, ia, :].astype(np.float64)    # [4, 32, 21, 128] (d,r,j,u)
    kb = K[:, :, ib, :].astype(np.float64)
    kc = K[:, :, ic, :].astype(np.float64)
    K3 = (
        ka[:, None, None] * kb[None, :, None] * kc[None, None, :]
    )                                          # [d0,d1,d2,r,j,u]
    K3 = K3.transpose(4, 0, 1, 2, 3, 5).reshape(NT, D3, R, U)
    K3f = np.zeros((NFAC, D3, R, U), dtype=np.float64)
    K3f[:NT] = K3
    K3f[NT, :D] = K[:, :, 63, :]
    K3f = K3f.transpose(0, 1, 3, 2)            # [j, d3, u, r]
    krs = []
    for uq in range(NU):
        sl = K3f[:, :, uq * ULOC : (uq + 1) * ULOC, :].reshape(NFAC, D3, RULOC)
        krs.append(
            np.ascontiguousarray(sl.reshape(NGRP, 2 * D3, RULOC), dtype=f32)
        )

    xts = []
    for cb in range(NB):
        Xc = X[cb * BLOC : (cb + 1) * BLOC].astype(np.float64)  # [512, 64, 4]
        xa = Xc[:, ia, :]
        xb = Xc[:, ib, :]
        xc = Xc[:, ic, :]
        X3 = (
            xa[:, :, :, None, None] * xb[:, :, None, :, None] * xc[:, :, None, None, :]
        ).reshape(BLOC, NT, D3)
        X3f = np.zeros((BLOC, NFAC, D3), dtype=np.float64)
        X3f[:, :NT] = X3
        X3f[:, NT, :D] = Xc[:, 63, :]
        xt = X3f.transpose(1, 2, 0).reshape(NGRP, 128, BLOC)   # [m, row, b]
        xts.append(
            np.ascontiguousarray(
                xt.transpose(1, 0, 2).reshape(128, NGRP * BLOC), dtype=f32
            )
        )
    return xts, krs


def kernel(**inputs):
    from concourse.bass_utils import run_bass_kernel_spmd

    X = np.asarray(inputs["X"], dtype=np.float32)
    K = np.asarray(inputs["kernel"], dtype=np.float32)
    assert X.shape == (B, F, D) and K.shape == (D, R, F, U)

    if "nc" not in _cached:
        _cached["nc"] = _build_nc()
    nc = _cached["nc"]

    xts, krs = _host_prep(X, K)
    in_maps = [
        {"xt": xts[c // NU], "kr": krs[c % NU]} for c in range(NCORES)
    ]
    res = run_bass_kernel_spmd(nc, in_maps, core_ids=list(range(NCORES)))
    out = np.empty((B, U), dtype=np.float32)
    for c in range(NCORES):
        cb, uq = c // NU, c % NU
        out[cb * BLOC : (cb + 1) * BLOC, uq * ULOC : (uq + 1) * ULOC] = (
            res.results[c]["out"]
        )
    return out
